# revision 1
# baseline (speedup 1.0000x reference)
"""Trainium2 Bass kernel for the RNN-T JointNetwork problem.

Computes log_softmax(tanh(cat(enc, pred)) @ W.T + b) over the vocab dim
for logits of shape [B=4, T=200, U=50, V=1024], fp32.

Strategy (data-parallel over the 800 flattened (b,t) rows, 100 per core):
  setup (per core, on device):
    teT  = tanh(encT_slice)          [512, 100]   (ACT)
    tpT  = tanh(predT_slice)         [512, 50]    (ACT)
    enc_p  = teT.T @ WeT             [100, 1024]  (PE, fp32)
    pred_b = tpT.T @ WpT + b         [50, 1024]   (PE, fp32)
  main loop over 40 row-tiles of 128 rows (row r = t*50 + u):
    x    = u_ind[k].T @ pred_b + t_ind[k].T @ enc_p   (PE -> PSUM, f32r;
           one-hot stationary operands do the broadcast-add, exactly)
    e,s  = exp(x), rowsum(e)         (ACT with accum_out)
    lse  = ln(s)                     (ACT)
    out  = x - lse                   (DVE tensor_scalar, PSUM -> SBUF)
    DMA out tile -> DRAM (round-robin over issue queues)
"""

import numpy as np

import concourse.bass as bass
import concourse.bacc as bacc
import concourse.tile as tile
from concourse import mybir
from concourse.bass_utils import run_bass_kernel_spmd

# Problem shapes (hardcoded per contract).
B, T, U, D, V = 4, 200, 50, 512, 1024
N_CORES = 8
BT = B * T                     # 800 flattened (b,t) rows
TPC = BT // N_CORES            # 100 (b,t) rows per core
ROWS = TPC * U                 # 5000 output rows per core
P = 128
NT = (ROWS + P - 1) // P       # 40 row-tiles per core
NV = V // 512                  # fp32 moving-operand free-dim limit is 512
DC = D // P                    # 4 contraction chunks of 128 for D=512

f32 = mybir.dt.float32
f32r = mybir.dt.float32r
bf16 = mybir.dt.bfloat16

# Which issue queues take the 40 output-tile DMAs, round-robin.
OUT_DMA_ENGINES = ("sync", "gpsimd")

TRACE = False
LAST_RESULT = None

_CACHE = {}


def _patch_act_tables():
    """Pin Exp/Ln to the one table set containing both, so the activation
    table-load pass never alternates sets inside the main loop.

    Claiming a set does NOT contain a function is always safe (it can only
    add loads); here it redirects Exp away from sets lacking Ln.
    """
    if getattr(bacc, "_joint_act_patch", False):
        return
    orig = bacc.get_activation_tables

    def patched(arch):
        t = dict(orig(arch))
        keep = "natural_log_exp_and_others"
        drop = {mybir.ActivationFunctionType.Exp, mybir.ActivationFunctionType.Ln}
        for name, fns in t.items():
            if name != keep:
                t[name] = set(fns) - drop
        return t

    bacc.get_activation_tables = patched
    bacc._joint_act_patch = True


def _build_indicators():
    """Per-row-tile one-hot stationary operands, shared by all cores.

    u_ind[u, k, c] = 1 iff row (128k+c) has u(row) == u  (row % 50)
    t_ind[t, k, c] = 1 iff row (128k+c) has t(row) == t  (row // 50)
    Columns for rows >= ROWS (tail of the last tile) are all-zero.
    """
    r = np.arange(NT * P)
    valid = r < ROWS
    u_ind = np.zeros((U, NT * P), dtype=np.float32)
    t_ind = np.zeros((TPC, NT * P), dtype=np.float32)
    u_ind[(r % U)[valid], r[valid]] = 1.0
    t_ind[(r // U)[valid], r[valid]] = 1.0
    return (
        np.ascontiguousarray(u_ind.reshape(U, NT, P)),
        np.ascontiguousarray(t_ind.reshape(TPC, NT, P)),
    )


def _build_program():
    _patch_act_tables()
    nc = bacc.Bacc("TRN2", target_bir_lowering=False, debug=False,
                   num_devices=N_CORES)

    encT = nc.dram_tensor("encT", [D, TPC], f32, kind="ExternalInput")
    predT = nc.dram_tensor("predT", [D, U], f32, kind="ExternalInput")
    # W in bf16: halves the 4MB load and runs setup matmuls at full PE rate;
    # the resulting ~1e-3 relative rounding of the logits is far inside the
    # output tolerance (outputs are O(1..10), fp32 pipeline elsewhere).
    wT = nc.dram_tensor("wT", [2 * D, V], bf16, kind="ExternalInput")
    bias = nc.dram_tensor("bias", [V], f32, kind="ExternalInput")
    u_ind = nc.dram_tensor("u_ind", [U, NT, P], bf16, kind="ExternalInput")
    t_ind = nc.dram_tensor("t_ind", [TPC, NT, P], bf16, kind="ExternalInput")
    out = nc.dram_tensor("out", [ROWS, V], f32, kind="ExternalOutput")

    with tile.TileContext(nc) as tc:
        with (
            tc.tile_pool(name="consts", bufs=1) as consts,
            tc.tile_pool(name="psum", bufs=4, space=bass.MemorySpace.PSUM) as psum,
            tc.tile_pool(name="scratch", bufs=2) as scratch,
            tc.tile_pool(name="outs", bufs=6) as outs,
            tc.tile_pool(name="small", bufs=8) as small,
        ):
            # ---- load constants / inputs (spread across DMA issue queues,
            #      wT chunked so setup matmuls can start before it finishes) ----
            wt_sb = consts.tile([P, 2 * DC, V], bf16)
            wT_r = wT.ap().rearrange("(c p) v -> p c v", p=P)
            for c in range(2 * DC):
                eng = nc.sync if c % 2 == 0 else nc.scalar
                eng.dma_start(out=wt_sb[:, c, :], in_=wT_r[:, c, :])
            encT_sb = consts.tile([P, DC, TPC], f32)
            nc.scalar.dma_start(out=encT_sb[:], in_=encT.ap().rearrange(
                "(c p) t -> p c t", p=P))
            predT_sb = consts.tile([P, DC, U], f32)
            nc.scalar.dma_start(out=predT_sb[:], in_=predT.ap().rearrange(
                "(c p) u -> p c u", p=P))
            # indicators split so the first tiles don't wait on the full 3MB
            KSPLIT = 6
            uind_sb = consts.tile([U, NT, P], bf16)
            nc.gpsimd.dma_start(out=uind_sb[:, :KSPLIT, :],
                                in_=u_ind.ap()[:, :KSPLIT, :])
            nc.gpsimd.dma_start(out=uind_sb[:, KSPLIT:, :],
                                in_=u_ind.ap()[:, KSPLIT:, :])
            tind_sb = consts.tile([TPC, NT, P], bf16)
            nc.gpsimd.dma_start(out=tind_sb[:, :KSPLIT, :],
                                in_=t_ind.ap()[:, :KSPLIT, :])
            nc.gpsimd.dma_start(out=tind_sb[:, KSPLIT:, :],
                                in_=t_ind.ap()[:, KSPLIT:, :])
            b_sb = consts.tile([1, V], f32)
            nc.scalar.dma_start(out=b_sb[:], in_=bias.ap().rearrange(
                "(p v) -> p v", p=1))
            ones_u = consts.tile([1, U], f32)
            nc.vector.memset(ones_u[:], 1.0)

            # ---- tanh of activations (transposed layout: d on partitions),
            #      bf16 out to pair with the bf16 weights in the setup GEMMs ----
            teT = consts.tile([P, DC, TPC], bf16)
            nc.scalar.activation(teT[:], encT_sb[:],
                                 mybir.ActivationFunctionType.Tanh)
            tpT = consts.tile([P, DC, U], bf16)
            nc.scalar.activation(tpT[:], predT_sb[:],
                                 mybir.ActivationFunctionType.Tanh)

            # ---- enc_p[t, v] = sum_d teT[d, t] * We[v, d] ----
            enc_p = consts.tile([TPC, V], bf16)
            enc_ps = psum.tile([TPC, V], f32, tag="x")
            for vc in range(NV):
                sl = slice(vc * 512, (vc + 1) * 512)
                for c in range(DC):
                    nc.tensor.matmul(enc_ps[:, sl], teT[:, c, :],
                                     wt_sb[:, c, sl],
                                     start=(c == 0), stop=(c == DC - 1))
            nc.vector.tensor_copy(enc_p[:], enc_ps[:])

            # ---- pred_b[u, v] = sum_d tpT[d, u] * Wp[v, d] + b[v] ----
            pred_b = consts.tile([U, V], bf16)
            pred_ps = psum.tile([U, V], f32, tag="x")
            for vc in range(NV):
                sl = slice(vc * 512, (vc + 1) * 512)
                for c in range(DC):
                    nc.tensor.matmul(pred_ps[:, sl], tpT[:, c, :],
                                     wt_sb[:, DC + c, sl],
                                     start=(c == 0), stop=False)
                nc.tensor.matmul(pred_ps[:, sl], ones_u[:], b_sb[:, sl],
                                 start=False, stop=True)
            nc.vector.tensor_copy(pred_b[:], pred_ps[:])

            # ---- main loop over row tiles ----
            dma_engines = [getattr(nc, e) for e in OUT_DMA_ENGINES]
            for k in range(NT):
                r0 = k * P
                rows = min(P, ROWS - r0)
                x_ps = psum.tile([P, V], f32, tag="x")
                # f32r: full-rate fp32 streaming on the PE.  The one-hot
                # stationary operand is exact in any precision; only the
                # pass-through of pred_b/enc_p values sees f32r rounding.
                for vc in range(NV):
                    sl = slice(vc * 512, (vc + 1) * 512)
                    nc.tensor.matmul(x_ps[:rows, sl],
                                     uind_sb[:, k, :rows],
                                     pred_b[:, sl],
                                     start=True, stop=False)
                for vc in range(NV):
                    sl = slice(vc * 512, (vc + 1) * 512)
                    nc.tensor.matmul(x_ps[:rows, sl],
                                     tind_sb[:, k, :rows],
                                     enc_p[:, sl],
                                     start=False, stop=True)
                sums = small.tile([P, 1], f32)
                escr = scratch.tile([P, V], f32)
                nc.scalar.activation(escr[:rows], x_ps[:rows],
                                     mybir.ActivationFunctionType.Exp,
                                     accum_out=sums[:rows])
                lse = small.tile([P, 1], f32)
                nc.scalar.activation(lse[:rows], sums[:rows],
                                     mybir.ActivationFunctionType.Ln)
                o = outs.tile([P, V], f32)
                nc.vector.tensor_scalar_sub(o[:rows], x_ps[:rows], lse[:rows])
                eng = dma_engines[k % len(dma_engines)]
                eng.dma_start(out=out.ap()[r0:r0 + rows, :], in_=o[:rows])

    nc.compile()
    return nc


def kernel(enc_out, pred_out, W, b):
    global LAST_RESULT
    enc_out = np.asarray(enc_out, dtype=np.float32)
    pred_out = np.asarray(pred_out, dtype=np.float32)
    W = np.asarray(W, dtype=np.float32)
    b = np.asarray(b, dtype=np.float32)

    if "nc" not in _CACHE:
        _CACHE["nc"] = _build_program()
        _CACHE["ind"] = _build_indicators()
    nc = _CACHE["nc"]
    u_ind, t_ind = _CACHE["ind"]

    import ml_dtypes
    wT = np.ascontiguousarray(W.T).astype(ml_dtypes.bfloat16)   # [2D, V]
    enc_flat = enc_out.reshape(BT, D)                 # [800, 512]

    in_maps = []
    for c in range(N_CORES):
        bt0 = c * TPC
        b_idx = bt0 // T
        in_maps.append({
            "encT": np.ascontiguousarray(enc_flat[bt0:bt0 + TPC].T),
            "predT": np.ascontiguousarray(pred_out[b_idx].T),
            "wT": wT,
            "bias": b,
            "u_ind": u_ind.astype(ml_dtypes.bfloat16),
            "t_ind": t_ind.astype(ml_dtypes.bfloat16),
        })

    res = run_bass_kernel_spmd(nc, in_maps, core_ids=list(range(N_CORES)),
                               trace=TRACE)
    LAST_RESULT = res
    full = np.concatenate([r["out"] for r in res.results], axis=0)
    return full.reshape(B, T, U, V)



# revision 5
# speedup vs baseline: 1.0931x; 1.0931x over previous
"""Trainium2 Bass kernel for the RNN-T JointNetwork problem.

Computes log_softmax(tanh(cat(enc, pred)) @ W.T + b) over the vocab dim
for logits of shape [B=4, T=200, U=50, V=1024], fp32.

Strategy (data-parallel over the 800 flattened (b,t) rows, 100 per core):
  setup (per core, on device):
    teT  = tanh(encT_slice)          [512, 100]   (ACT)
    tpT  = tanh(predT_slice)         [512, 50]    (ACT)
    enc_p  = teT.T @ WeT             [100, 1024]  (PE)
    pred_b = tpT.T @ WpT + b         [50, 1024]   (PE)
    lse    = ln(exp(enc_p) @ exp(pred_b).T)  [100, 50]
             -- exact: sum_v e^{a+b} = (e^a)·(e^b) summed = one tiny GEMM,
             done via PE transposes of enc_p/pred_b chunks + 8 matmuls.
    -lse flattened row-major and re-tiled to column layout [128, 40]
             via a DRAM round-trip (affine reshapes).
  main loop over 40 row-tiles of 128 rows (row r = t*50 + u):
    mov[50:54] <- enc_p rows t0(k)..t0(k)+3     (SBUF-to-SBUF DMA)
    x = stat[k].T @ mov      ONE matmul per 512-col slice: stationary
        holds u-one-hot rows 0-49 plus local-t one-hot rows 50-53, so
        the broadcast add pred_b[u]+enc_p[t] is a single accumulation.
    o = x - lse  fused into the PSUM->SBUF eviction: ACT Copy with
        per-partition bias (even k) / DVE tensor_scalar_add (odd k).
    DMA out tile -> DRAM (gpsimd / scalar queues alternating).
"""

import numpy as np

import concourse.bass as bass
import concourse.bacc as bacc
import concourse.tile as tile
from concourse import mybir
from concourse.bass_utils import run_bass_kernel_spmd

# Problem shapes (hardcoded per contract).
B, T, U, D, V = 4, 200, 50, 512, 1024
N_CORES = 8
BT = B * T                     # 800 flattened (b,t) rows
TPC = BT // N_CORES            # 100 (b,t) rows per core
ROWS = TPC * U                 # 5000 output rows per core
P = 128
NT = (ROWS + P - 1) // P       # 40 row-tiles per core
NV = V // 512                  # PSUM bank = 512 fp32 cols per matmul
DC = D // P                    # 4 contraction chunks of 128 for D=512
VC = V // P                    # 8 vocab chunks of 128 (lse transposes)
NK = U + 4                     # stationary rows: 50 u-one-hot + 4 local-t

f32 = mybir.dt.float32
bf16 = mybir.dt.bfloat16

# t0c[k]: first enc_p row staged for tile k (clamped so t0c+4 <= TPC).
T0C = [min((k * P) // U, TPC - 4) for k in range(NT)]

TRACE = False
LAST_RESULT = None

_CACHE = {}


def _patch_act_tables():
    """Pin Exp/Ln to the one table set containing both, so the activation
    table-load pass never alternates sets (Copy lives in every set)."""
    if getattr(bacc, "_joint_act_patch", False):
        return
    orig = bacc.get_activation_tables

    def patched(arch):
        t = dict(orig(arch))
        keep = "natural_log_exp_and_others"
        drop = {mybir.ActivationFunctionType.Exp, mybir.ActivationFunctionType.Ln}
        for name, fns in t.items():
            if name != keep:
                t[name] = set(fns) - drop
        return t

    bacc.get_activation_tables = patched
    bacc._joint_act_patch = True


def _build_stationary():
    """Per-row-tile combined one-hot stationary [NK, NT, P], shared by all
    cores.  Column c of tile k maps to row r = 128k + c:
      row u(r)            gets 1.0  (selects pred_b row)
      row 50 + t(r)-t0cated gets 1.0  (selects the staged enc_p row)
    Columns for rows >= ROWS are all-zero."""
    r = np.arange(NT * P)
    valid = r < ROWS
    k_of = r // P
    c_of = r % P
    u = r % U
    t = r // U
    off = t - np.asarray(T0C)[k_of]
    assert ((off[valid] >= 0) & (off[valid] <= 3)).all()
    stat = np.zeros((NK, NT, P), dtype=np.float32)
    stat[u[valid], k_of[valid], c_of[valid]] = 1.0
    stat[(U + off)[valid], k_of[valid], c_of[valid]] = 1.0
    return stat


def _build_program():
    _patch_act_tables()
    nc = bacc.Bacc("TRN2", target_bir_lowering=False, debug=False,
                   num_devices=N_CORES)

    encT = nc.dram_tensor("encT", [D, TPC], f32, kind="ExternalInput")
    predT = nc.dram_tensor("predT", [D, U], f32, kind="ExternalInput")
    # W in bf16: halves the load and runs matmuls at full PE rate; ~1e-3
    # relative rounding of the logits, far inside tolerance.
    wT = nc.dram_tensor("wT", [2 * D, V], bf16, kind="ExternalInput")
    bias = nc.dram_tensor("bias", [1, V], bf16, kind="ExternalInput")
    stat = nc.dram_tensor("stat", [NK, NT, P], bf16, kind="ExternalInput")
    ident = nc.dram_tensor("ident", [P, P], bf16, kind="ExternalInput")
    scratch = nc.dram_tensor("lse_scratch", [NT * P], f32, kind="Internal")
    out = nc.dram_tensor("out", [ROWS, V], f32, kind="ExternalOutput")

    ACT = mybir.ActivationFunctionType

    with tile.TileContext(nc) as tc:
        with (
            tc.tile_pool(name="consts", bufs=1) as consts,
            tc.tile_pool(name="pbig", bufs=3, space=bass.MemorySpace.PSUM) as pbig,
            tc.tile_pool(name="psmall", bufs=2, space=bass.MemorySpace.PSUM) as psmall,
            tc.tile_pool(name="outs", bufs=8) as outs,
        ):
            # ---- input DMAs: encT + We on the scalar queue (tanh + enc
            #      GEMM are the critical path), predT/ident/bias/Wp on sync,
            #      stationary one-hots on gpsimd ----
            encT_sb = consts.tile([P, DC, TPC], f32)
            nc.scalar.dma_start(out=encT_sb[:], in_=encT.ap().rearrange(
                "(c p) t -> p c t", p=P))
            predT_sb = consts.tile([P, DC, U], f32)
            nc.sync.dma_start(out=predT_sb[:], in_=predT.ap().rearrange(
                "(c p) u -> p c u", p=P))
            ident_sb = consts.tile([P, P], bf16)
            nc.sync.dma_start(out=ident_sb[:], in_=ident.ap())
            b_sb = consts.tile([1, V], bf16)
            nc.sync.dma_start(out=b_sb[:], in_=bias.ap())
            wt_sb = consts.tile([P, 2 * DC, V], bf16)
            wT_r = wT.ap().rearrange("(c p) v -> p c v", p=P)
            for c in range(2 * DC):
                eng = nc.scalar if c < DC else nc.sync
                eng.dma_start(out=wt_sb[:, c, :], in_=wT_r[:, c, :])
            KS = 4
            stat_sb = consts.tile([NK, NT, P], bf16)
            nc.gpsimd.dma_start(out=stat_sb[:, :KS, :], in_=stat.ap()[:, :KS, :])
            nc.gpsimd.dma_start(out=stat_sb[:, KS:, :], in_=stat.ap()[:, KS:, :])
            ones1 = consts.tile([1, U], bf16)
            nc.vector.memset(ones1[:], 1.0)

            # ---- tanh of activations (d on partitions), bf16 out ----
            teT = consts.tile([P, DC, TPC], bf16)
            nc.scalar.activation(teT[:], encT_sb[:], ACT.Tanh)
            tpT = consts.tile([P, DC, U], bf16)
            nc.scalar.activation(tpT[:], predT_sb[:], ACT.Tanh)

            # ---- enc_p[t, v] = sum_d teT[d, t] * We[v, d] ----
            enc_ps = pbig.tile([TPC, V], f32, tag="x")
            for vc in range(NV):
                sl = slice(vc * 512, (vc + 1) * 512)
                for c in range(DC):
                    nc.tensor.matmul(enc_ps[:, sl], teT[:, c, :],
                                     wt_sb[:, c, sl],
                                     start=(c == 0), stop=(c == DC - 1))
            enc_p = consts.tile([TPC, V], bf16)
            nc.vector.tensor_copy(enc_p[:], enc_ps[:])

            # ---- pred_b[u, v] = sum_d tpT[d, u] * Wp[v, d] + b[v] ----
            pred_ps = pbig.tile([U, V], f32, tag="x")
            for vc in range(NV):
                sl = slice(vc * 512, (vc + 1) * 512)
                for c in range(DC):
                    nc.tensor.matmul(pred_ps[:, sl], tpT[:, c, :],
                                     wt_sb[:, DC + c, sl],
                                     start=(c == 0), stop=False)
                nc.tensor.matmul(pred_ps[:, sl], ones1[:], b_sb[:, sl],
                                 start=False, stop=True)
            # Three moving buffers, each: rows 0-49 pred_b (constant),
            # rows 50-53 per-tile enc_p rows (rewritten in the main loop).
            movs = [consts.tile([NK, V], bf16, name=f"mov{i}")
                    for i in range(3)]
            for m in movs:
                nc.vector.tensor_copy(m[0:U, :], pred_ps[:])

            # ---- lse[t, u] = ln(sum_v exp(enc_p) * exp(pred_b)) ----
            Etr = consts.tile([P, VC, TPC], bf16)
            for c in range(VC):
                pt = psmall.tile([P, TPC], bf16, tag="t")
                nc.tensor.transpose(pt[:], enc_p[:, c * P:(c + 1) * P],
                                    ident_sb[:TPC, :TPC])
                nc.scalar.activation(Etr[:, c, :], pt[:], ACT.Exp)
            Ptr = consts.tile([P, VC, U], bf16)
            for c in range(VC):
                pt = psmall.tile([P, U], bf16, tag="t")
                nc.tensor.transpose(pt[:], movs[0][0:U, c * P:(c + 1) * P],
                                    ident_sb[:U, :U])
                nc.scalar.activation(Ptr[:, c, :], pt[:], ACT.Exp)
            S_ps = psmall.tile([TPC, U], f32, tag="t")
            for c in range(VC):
                nc.tensor.matmul(S_ps[:], Etr[:, c, :], Ptr[:, c, :],
                                 start=(c == 0), stop=(c == VC - 1))
            lse = consts.tile([TPC, U], f32)
            nc.scalar.activation(lse[:], S_ps[:], ACT.Ln)
            nlse = consts.tile([TPC, U], f32)
            nc.vector.tensor_scalar_mul(nlse[:], lse[:], -1.0)
            # Flatten [100, 50] (row-major = row index r) into column
            # layout [128, 40] via a DRAM round-trip; tail rows of
            # scratch are never-read garbage (pad rows only).
            nc.scalar.dma_start(
                out=scratch.ap()[0:ROWS].rearrange("(t u) -> t u", t=TPC),
                in_=nlse[:])
            ncols = consts.tile([P, NT], f32)
            nc.scalar.dma_start(
                out=ncols[:], in_=scratch.ap().rearrange("(k p) -> p k", p=P))

            # ---- main loop over row tiles ----
            for k in range(NT):
                r0 = k * P
                rows = min(P, ROWS - r0)
                t0 = T0C[k]
                mov = movs[k % 3]
                nc.sync.dma_start(out=mov[U:U + 4, :], in_=enc_p[t0:t0 + 4, :])
                x_ps = pbig.tile([P, V], f32, tag="x")
                for vc in range(NV):
                    sl = slice(vc * 512, (vc + 1) * 512)
                    nc.tensor.matmul(x_ps[:rows, sl],
                                     stat_sb[:, k, :rows],
                                     mov[:, sl],
                                     start=True, stop=True)
                o = outs.tile([P, V], f32)
                if k % 2 == 0:
                    nc.scalar.activation(o[:rows], x_ps[:rows], ACT.Identity,
                                         bias=ncols[:rows, k:k + 1])
                else:
                    nc.vector.tensor_scalar_add(o[:rows], x_ps[:rows],
                                                ncols[:rows, k:k + 1])
                eng = nc.gpsimd if k % 2 == 0 else nc.scalar
                eng.dma_start(out=out.ap()[r0:r0 + rows, :], in_=o[:rows])

    nc.compile()
    return nc


def kernel(enc_out, pred_out, W, b):
    global LAST_RESULT
    enc_out = np.asarray(enc_out, dtype=np.float32)
    pred_out = np.asarray(pred_out, dtype=np.float32)
    W = np.asarray(W, dtype=np.float32)
    b = np.asarray(b, dtype=np.float32)

    if "nc" not in _CACHE:
        _CACHE["nc"] = _build_program()
        _CACHE["stat"] = _build_stationary()
    nc = _CACHE["nc"]

    import ml_dtypes
    wT = np.ascontiguousarray(W.T).astype(ml_dtypes.bfloat16)   # [2D, V]
    stat = _CACHE["stat"].astype(ml_dtypes.bfloat16)
    ident = np.eye(P, dtype=ml_dtypes.bfloat16)
    bias = np.ascontiguousarray(b.reshape(1, V)).astype(ml_dtypes.bfloat16)
    enc_flat = enc_out.reshape(BT, D)                 # [800, 512]

    in_maps = []
    for c in range(N_CORES):
        bt0 = c * TPC
        b_idx = bt0 // T
        in_maps.append({
            "encT": np.ascontiguousarray(enc_flat[bt0:bt0 + TPC].T),
            "predT": np.ascontiguousarray(pred_out[b_idx].T),
            "wT": wT,
            "bias": bias,
            "stat": stat,
            "ident": ident,
        })

    res = run_bass_kernel_spmd(nc, in_maps, core_ids=list(range(N_CORES)),
                               trace=TRACE)
    LAST_RESULT = res
    full = np.concatenate([r["out"] for r in res.results], axis=0)
    return full.reshape(B, T, U, V)


# revision 6
# speedup vs baseline: 1.1290x; 1.0329x over previous
"""Trainium2 Bass kernel for the RNN-T JointNetwork problem.

Computes log_softmax(tanh(cat(enc, pred)) @ W.T + b) over the vocab dim
for logits of shape [B=4, T=200, U=50, V=1024], fp32.

Strategy (data-parallel over the 800 flattened (b,t) rows, 100 per core):
  setup (per core, on device):
    teT  = tanh(encT_slice)          [512, 100]   (ACT)
    tpT  = tanh(predT_slice)         [512, 50]    (ACT)
    enc_p  = teT.T @ WeT             [100, 1024]  (PE)
    pred_b = tpT.T @ WpT + b         [50, 1024]   (PE)
    lse    = ln(exp(enc_p) @ exp(pred_b).T)  [100, 50]
             -- exact: sum_v e^{a+b} = (e^a)·(e^b) summed = one tiny GEMM,
             done via PE transposes of enc_p/pred_b chunks + 8 matmuls.
    -lse flattened row-major and re-tiled to column layout [128, 40]
             via a DRAM round-trip (split 2x2 across queues for latency).
  main loop over 40 row-tiles of 128 rows (row r = t*50 + u):
    mov[50:54] <- enc_p rows t0(k)..t0(k)+3     (SBUF-to-SBUF DMA)
    x = stat[k].T @ mov      ONE matmul per 512-col slice: stationary
        holds u-one-hot rows 0-49 plus local-t one-hot rows 50-53, so
        the broadcast add pred_b[u]+enc_p[t] is a single accumulation.
    o = x - lse  fused into the PSUM->SBUF eviction, split per tile:
        ACT Identity+bias on cols 0-511, DVE tensor_scalar_add on 512+.
    DMA out tile -> DRAM (3 queues round-robin).

All input DMAs use (p c) row-interleaved layouts so each partition reads
one large contiguous DRAM block (the d-contraction order is a consistent
permutation on both W and the activations, so results are unchanged).
"""

import numpy as np

import concourse.bass as bass
import concourse.bacc as bacc
import concourse.tile as tile
from concourse import mybir
from concourse.bass_utils import run_bass_kernel_spmd

# Problem shapes (hardcoded per contract).
B, T, U, D, V = 4, 200, 50, 512, 1024
N_CORES = 8
BT = B * T                     # 800 flattened (b,t) rows
TPC = BT // N_CORES            # 100 (b,t) rows per core
ROWS = TPC * U                 # 5000 output rows per core
P = 128
NT = (ROWS + P - 1) // P       # 40 row-tiles per core
NV = V // 512                  # PSUM bank = 512 fp32 cols per matmul
DC = D // P                    # 4 contraction chunks of 128 for D=512
VC = V // P                    # 8 vocab chunks of 128 (lse transposes)
NK = U + 4                     # stationary rows: 50 u-one-hot + 4 local-t
NMOV = 4                       # rotating moving-operand buffers

f32 = mybir.dt.float32
bf16 = mybir.dt.bfloat16

# t0c[k]: first enc_p row staged for tile k (clamped so t0c+4 <= TPC).
T0C = [min((k * P) // U, TPC - 4) for k in range(NT)]

TRACE = False
LAST_RESULT = None

_CACHE = {}


def _patch_act_tables():
    """Pin Exp/Ln to the one table set containing both, so the activation
    table-load pass never alternates sets (Identity lives in every set)."""
    if getattr(bacc, "_joint_act_patch", False):
        return
    orig = bacc.get_activation_tables

    def patched(arch):
        t = dict(orig(arch))
        keep = "natural_log_exp_and_others"
        drop = {mybir.ActivationFunctionType.Exp, mybir.ActivationFunctionType.Ln}
        for name, fns in t.items():
            if name != keep:
                t[name] = set(fns) - drop
        return t

    bacc.get_activation_tables = patched
    bacc._joint_act_patch = True


def _build_stationary():
    """Per-row-tile combined one-hot stationary [NK, NT, P], shared by all
    cores.  Column c of tile k maps to row r = 128k + c:
      row u(r)             gets 1.0  (selects pred_b row)
      row 50 + t(r)-t0c[k] gets 1.0  (selects the staged enc_p row)
    Columns for rows >= ROWS are all-zero."""
    r = np.arange(NT * P)
    valid = r < ROWS
    k_of = r // P
    c_of = r % P
    u = r % U
    t = r // U
    off = t - np.asarray(T0C)[k_of]
    assert ((off[valid] >= 0) & (off[valid] <= 3)).all()
    stat = np.zeros((NK, NT, P), dtype=np.float32)
    stat[u[valid], k_of[valid], c_of[valid]] = 1.0
    stat[(U + off)[valid], k_of[valid], c_of[valid]] = 1.0
    return stat


def _build_program():
    _patch_act_tables()
    nc = bacc.Bacc("TRN2", target_bir_lowering=False, debug=False,
                   num_devices=N_CORES)

    encT = nc.dram_tensor("encT", [D, TPC], f32, kind="ExternalInput")
    predT = nc.dram_tensor("predT", [D, U], f32, kind="ExternalInput")
    # W in bf16, split into enc/pred halves: halves the load and runs
    # matmuls at full PE rate; ~1e-3 relative rounding, inside tolerance.
    wTe = nc.dram_tensor("wTe", [D, V], bf16, kind="ExternalInput")
    wTp = nc.dram_tensor("wTp", [D, V], bf16, kind="ExternalInput")
    bias = nc.dram_tensor("bias", [1, V], bf16, kind="ExternalInput")
    stat = nc.dram_tensor("stat", [NK, NT, P], bf16, kind="ExternalInput")
    ident = nc.dram_tensor("ident", [P, P], bf16, kind="ExternalInput")
    scratch = nc.dram_tensor("lse_scratch", [NT * P], f32, kind="Internal")
    out = nc.dram_tensor("out", [ROWS, V], f32, kind="ExternalOutput")

    ACT = mybir.ActivationFunctionType

    with tile.TileContext(nc) as tc:
        with (
            tc.tile_pool(name="consts", bufs=1) as consts,
            tc.tile_pool(name="ps", bufs=8, space=bass.MemorySpace.PSUM) as ps,
            tc.tile_pool(name="outs", bufs=8) as outs,
        ):
            # ---- input DMAs.  (p c) interleave: partition p reads rows
            #      [c*p .. c*p+c) as one contiguous DRAM block. ----
            encT_sb = consts.tile([P, DC, TPC], f32)
            nc.scalar.dma_start(out=encT_sb[:], in_=encT.ap().rearrange(
                "(p c) t -> p c t", p=P))
            predT_sb = consts.tile([P, DC, U], f32)
            nc.sync.dma_start(out=predT_sb[:], in_=predT.ap().rearrange(
                "(p c) u -> p c u", p=P))
            ident_sb = consts.tile([P, P], bf16)
            nc.sync.dma_start(out=ident_sb[:], in_=ident.ap())
            b_sb = consts.tile([1, V], bf16)
            nc.sync.dma_start(out=b_sb[:], in_=bias.ap())
            wt_e = consts.tile([P, DC, V], bf16)
            wTe_r = wTe.ap().rearrange("(p c) v -> p c v", p=P)
            wt_p = consts.tile([P, DC, V], bf16)
            wTp_r = wTp.ap().rearrange("(p c) v -> p c v", p=P)
            for c in range(DC):
                nc.scalar.dma_start(out=wt_e[:, c, :], in_=wTe_r[:, c, :])
            for c in range(DC):
                nc.sync.dma_start(out=wt_p[:, c, :], in_=wTp_r[:, c, :])
            stat_sb = consts.tile([NK, NT, P], bf16)
            nc.gpsimd.dma_start(out=stat_sb[:], in_=stat.ap())
            ones1 = consts.tile([1, U], bf16)
            nc.vector.memset(ones1[:], 1.0)

            # ---- tanh of activations (d on partitions), bf16 out ----
            teT = consts.tile([P, DC, TPC], bf16)
            nc.scalar.activation(teT[:], encT_sb[:], ACT.Tanh)
            tpT = consts.tile([P, DC, U], bf16)
            nc.scalar.activation(tpT[:], predT_sb[:], ACT.Tanh)

            # ---- enc_p[t, v] = sum_d teT[d, t] * We[v, d]  (2 halves) ----
            enc_p = consts.tile([TPC, V], bf16)
            for vc in range(NV):
                sl = slice(vc * 512, (vc + 1) * 512)
                eh = ps.tile([TPC, 512], f32, tag="ps", name=f"encps{vc}")
                for c in range(DC):
                    nc.tensor.matmul(eh[:], teT[:, c, :], wt_e[:, c, sl],
                                     start=(c == 0), stop=(c == DC - 1))
                nc.vector.tensor_copy(enc_p[:, sl], eh[:])

            # ---- pred_b[u, v] = sum_d tpT[d, u] * Wp[v, d] + b[v] ----
            movs = [consts.tile([NK, V], bf16, name=f"mov{i}")
                    for i in range(NMOV)]
            for vc in range(NV):
                sl = slice(vc * 512, (vc + 1) * 512)
                ph = ps.tile([U, 512], f32, tag="ps", name=f"predps{vc}")
                for c in range(DC):
                    nc.tensor.matmul(ph[:], tpT[:, c, :], wt_p[:, c, sl],
                                     start=(c == 0), stop=False)
                nc.tensor.matmul(ph[:], ones1[:], b_sb[:, sl],
                                 start=False, stop=True)
                for m in movs:
                    nc.vector.tensor_copy(m[0:U, sl], ph[:])

            # ---- lse[t, u] = ln(sum_v exp(enc_p) * exp(pred_b)) ----
            Etr = consts.tile([P, VC, TPC], bf16)
            for c in range(VC):
                pt = ps.tile([P, TPC], bf16, tag="ps", name=f"ept{c}")
                nc.tensor.transpose(pt[:], enc_p[:, c * P:(c + 1) * P],
                                    ident_sb[:TPC, :TPC])
                nc.scalar.activation(Etr[:, c, :], pt[:], ACT.Exp)
            Ptr = consts.tile([P, VC, U], bf16)
            for c in range(VC):
                pt = ps.tile([P, U], bf16, tag="ps", name=f"ppt{c}")
                nc.tensor.transpose(pt[:], movs[0][0:U, c * P:(c + 1) * P],
                                    ident_sb[:U, :U])
                nc.scalar.activation(Ptr[:, c, :], pt[:], ACT.Exp)
            S_ps = ps.tile([TPC, U], f32, tag="ps")
            for c in range(VC):
                nc.tensor.matmul(S_ps[:], Etr[:, c, :], Ptr[:, c, :],
                                 start=(c == 0), stop=(c == VC - 1))
            lse = consts.tile([TPC, U], f32)
            nc.scalar.activation(lse[:], S_ps[:], ACT.Ln)
            nlse = consts.tile([TPC, U], f32)
            nc.vector.tensor_scalar_mul(nlse[:], lse[:], -1.0)
            # Flatten [100, 50] (row-major = row index r) into column
            # layout [128, 40] via a DRAM round-trip, split across queues
            # so the first half lands early; scratch tail rows are
            # never-read garbage (pad rows only).
            RS = 52 * U                          # flat split at t=52
            nc.sync.dma_start(
                out=scratch.ap()[0:RS].rearrange("(t u) -> t u", t=52),
                in_=nlse[0:52, :])
            nc.gpsimd.dma_start(
                out=scratch.ap()[RS:ROWS].rearrange("(t u) -> t u", t=48),
                in_=nlse[52:TPC, :])
            sc_r = scratch.ap().rearrange("(k p) -> p k", p=P)
            ncols = consts.tile([P, NT], f32)
            nc.scalar.dma_start(out=ncols[:, 0:20], in_=sc_r[:, 0:20])
            nc.gpsimd.dma_start(out=ncols[:, 20:NT], in_=sc_r[:, 20:NT])

            # ---- main loop over row tiles ----
            out_engs = (nc.gpsimd, nc.scalar, nc.sync)
            for k in range(NT):
                r0 = k * P
                rows = min(P, ROWS - r0)
                t0 = T0C[k]
                mov = movs[k % NMOV]
                nc.sync.dma_start(out=mov[U:U + 4, :], in_=enc_p[t0:t0 + 4, :])
                nb = ncols[:rows, k:k + 1]
                o = outs.tile([P, V], f32)
                x0 = ps.tile([P, 512], f32, tag="ps")
                nc.tensor.matmul(x0[:rows], stat_sb[:, k, :rows],
                                 mov[:, 0:512], start=True, stop=True)
                x1 = ps.tile([P, 512], f32, tag="ps")
                nc.tensor.matmul(x1[:rows], stat_sb[:, k, :rows],
                                 mov[:, 512:V], start=True, stop=True)
                nc.scalar.activation(o[:rows, 0:512], x0[:rows], ACT.Identity,
                                     bias=nb)
                nc.vector.tensor_scalar_add(o[:rows, 512:V], x1[:rows], nb)
                eng = out_engs[k % 3]
                eng.dma_start(out=out.ap()[r0:r0 + rows, :], in_=o[:rows])

    nc.compile()
    return nc


def kernel(enc_out, pred_out, W, b):
    global LAST_RESULT
    enc_out = np.asarray(enc_out, dtype=np.float32)
    pred_out = np.asarray(pred_out, dtype=np.float32)
    W = np.asarray(W, dtype=np.float32)
    b = np.asarray(b, dtype=np.float32)

    if "nc" not in _CACHE:
        _CACHE["nc"] = _build_program()
        _CACHE["stat"] = _build_stationary()
    nc = _CACHE["nc"]

    import ml_dtypes
    wT = np.ascontiguousarray(W.T).astype(ml_dtypes.bfloat16)   # [2D, V]
    wTe = np.ascontiguousarray(wT[:D])
    wTp = np.ascontiguousarray(wT[D:])
    stat = _CACHE["stat"].astype(ml_dtypes.bfloat16)
    ident = np.eye(P, dtype=ml_dtypes.bfloat16)
    bias = np.ascontiguousarray(b.reshape(1, V)).astype(ml_dtypes.bfloat16)
    enc_flat = enc_out.reshape(BT, D)                 # [800, 512]

    in_maps = []
    for c in range(N_CORES):
        bt0 = c * TPC
        b_idx = bt0 // T
        in_maps.append({
            "encT": np.ascontiguousarray(enc_flat[bt0:bt0 + TPC].T),
            "predT": np.ascontiguousarray(pred_out[b_idx].T),
            "wTe": wTe,
            "wTp": wTp,
            "bias": bias,
            "stat": stat,
            "ident": ident,
        })

    res = run_bass_kernel_spmd(nc, in_maps, core_ids=list(range(N_CORES)),
                               trace=TRACE)
    LAST_RESULT = res
    full = np.concatenate([r["out"] for r in res.results], axis=0)
    return full.reshape(B, T, U, V)


# revision 11
# speedup vs baseline: 1.1514x; 1.0198x over previous
"""Trainium2 Bass kernel for the RNN-T JointNetwork problem.

Computes log_softmax(tanh(cat(enc, pred)) @ W.T + b) over the vocab dim
for logits of shape [B=4, T=200, U=50, V=1024], fp32.

Strategy (data-parallel over the 800 flattened (b,t) rows, 100 per core):
  setup (per core, on device):
    teT  = tanh(encT_slice)          [512, 100]   (ACT)
    tpT  = tanh(predT_slice)         [512, 50]    (ACT)
    enc_p  = teT.T @ WeT             [100, 1024]  (PE)
    pred_b = tpT.T @ WpT + b         [50, 1024]   (PE)
    lse    = ln(exp(enc_p) @ exp(pred_b).T)  [100, 50]
             -- exact: sum_v e^{a+b} = (e^a)·(e^b) summed = one tiny GEMM,
             done via PE transposes of enc_p/pred_b chunks + 8 matmuls.
    -lse flattened row-major and re-tiled to column layout [128, 40]
             via a DRAM round-trip (split 2x2 across queues for latency).
  main loop over 40 row-tiles of 128 rows (row r = t*50 + u):
    mov[50:54] <- enc_p rows t0(k)..t0(k)+3     (SBUF-to-SBUF DMA)
    x = stat[k].T @ mov      ONE matmul per 512-col slice: stationary
        holds u-one-hot rows 0-49 plus local-t one-hot rows 50-53, so
        the broadcast add pred_b[u]+enc_p[t] is a single accumulation.
    o = x - lse  fused into the PSUM->SBUF eviction, split per tile:
        ACT Identity+bias on cols 0-511, DVE tensor_scalar_add on 512+.
    DMA out tile -> DRAM (3 queues round-robin).

All input DMAs use (p c) row-interleaved layouts so each partition reads
one large contiguous DRAM block (the d-contraction order is a consistent
permutation on both W and the activations, so results are unchanged).
"""

import numpy as np

import concourse.bass as bass
import concourse.bacc as bacc
import concourse.tile as tile
from concourse import mybir
from concourse.bass_utils import run_bass_kernel_spmd

# Problem shapes (hardcoded per contract).
B, T, U, D, V = 4, 200, 50, 512, 1024
N_CORES = 8
BT = B * T                     # 800 flattened (b,t) rows
TPC = BT // N_CORES            # 100 (b,t) rows per core
ROWS = TPC * U                 # 5000 output rows per core
P = 128
NT = (ROWS + P - 1) // P       # 40 row-tiles per core
NV = V // 512                  # PSUM bank = 512 fp32 cols per matmul
DC = D // P                    # 4 contraction chunks of 128 for D=512
VC = V // P                    # 8 vocab chunks of 128 (lse transposes)
NK = U + 4                     # stationary rows: 50 u-one-hot + 4 local-t
NMOV = 4                       # rotating moving-operand buffers

f32 = mybir.dt.float32
bf16 = mybir.dt.bfloat16

# t0c[k]: first enc_p row staged for tile k (clamped so t0c+4 <= TPC).
T0C = [min((k * P) // U, TPC - 4) for k in range(NT)]

TRACE = False
LAST_RESULT = None

_CACHE = {}


def _patch_act_tables():
    """Pin Exp/Ln to the one table set containing both, so the activation
    table-load pass never alternates sets (Identity lives in every set)."""
    if getattr(bacc, "_joint_act_patch", False):
        return
    orig = bacc.get_activation_tables

    def patched(arch):
        t = dict(orig(arch))
        keep = "natural_log_exp_and_others"
        drop = {mybir.ActivationFunctionType.Exp, mybir.ActivationFunctionType.Ln}
        for name, fns in t.items():
            if name != keep:
                t[name] = set(fns) - drop
        return t

    bacc.get_activation_tables = patched
    bacc._joint_act_patch = True


def _build_stationary():
    """Per-row-tile combined one-hot stationary [NK, NT, P], shared by all
    cores.  Column c of tile k maps to row r = 128k + c:
      row u(r)             gets 1.0  (selects pred_b row)
      row 50 + t(r)-t0c[k] gets 1.0  (selects the staged enc_p row)
    Columns for rows >= ROWS are all-zero."""
    r = np.arange(NT * P)
    valid = r < ROWS
    k_of = r // P
    c_of = r % P
    u = r % U
    t = r // U
    off = t - np.asarray(T0C)[k_of]
    assert ((off[valid] >= 0) & (off[valid] <= 3)).all()
    stat = np.zeros((NK, NT, P), dtype=np.float32)
    stat[u[valid], k_of[valid], c_of[valid]] = 1.0
    stat[(U + off)[valid], k_of[valid], c_of[valid]] = 1.0
    return stat


def _build_program():
    _patch_act_tables()
    nc = bacc.Bacc("TRN2", target_bir_lowering=False, debug=False,
                   num_devices=N_CORES)

    encT = nc.dram_tensor("encT", [D, TPC], f32, kind="ExternalInput")
    predT = nc.dram_tensor("predT", [D, U], f32, kind="ExternalInput")
    # W in bf16, split into enc/pred halves: halves the load and runs
    # matmuls at full PE rate; ~1e-3 relative rounding, inside tolerance.
    wTe = nc.dram_tensor("wTe", [D, V], bf16, kind="ExternalInput")
    wTp = nc.dram_tensor("wTp", [D, V], bf16, kind="ExternalInput")
    bias = nc.dram_tensor("bias", [1, V], bf16, kind="ExternalInput")
    stat = nc.dram_tensor("stat", [NK, NT, P], bf16, kind="ExternalInput")
    ident = nc.dram_tensor("ident", [P, P], bf16, kind="ExternalInput")
    scratch = nc.dram_tensor("lse_scratch", [NT * P], f32, kind="Internal")
    out = nc.dram_tensor("out", [ROWS, V], f32, kind="ExternalOutput")

    ACT = mybir.ActivationFunctionType

    with tile.TileContext(nc) as tc:
        with (
            tc.tile_pool(name="consts", bufs=1) as consts,
            tc.tile_pool(name="ps", bufs=8, space=bass.MemorySpace.PSUM) as ps,
            tc.tile_pool(name="outs", bufs=8) as outs,
        ):
            # ---- input DMAs.  (p c) interleave: partition p reads rows
            #      [p*c .. p*c+c) as one contiguous DRAM block, so each W
            #      tensor is a single 128x8KB-packet DMA. ----
            encT_sb = consts.tile([P, DC, TPC], f32)
            nc.scalar.dma_start(out=encT_sb[:], in_=encT.ap().rearrange(
                "(p c) t -> p c t", p=P))
            predT_sb = consts.tile([P, DC, U], f32)
            nc.scalar.dma_start(out=predT_sb[:], in_=predT.ap().rearrange(
                "(p c) u -> p c u", p=P))
            ident_sb = consts.tile([P, P], bf16)
            nc.sync.dma_start(out=ident_sb[:], in_=ident.ap())
            b_sb = consts.tile([1, V], bf16)
            nc.sync.dma_start(out=b_sb[:], in_=bias.ap())
            wt_e = consts.tile([P, DC, V], bf16)
            nc.scalar.dma_start(out=wt_e[:], in_=wTe.ap().rearrange(
                "(p c) v -> p c v", p=P))
            wt_p = consts.tile([P, DC, V], bf16)
            nc.sync.dma_start(out=wt_p[:], in_=wTp.ap().rearrange(
                "(p c) v -> p c v", p=P))
            stat_sb = consts.tile([NK, NT, P], bf16)
            nc.gpsimd.dma_start(out=stat_sb[:], in_=stat.ap())
            ones1 = consts.tile([1, U], bf16)
            nc.vector.memset(ones1[:], 1.0)

            # ---- tanh of activations (d on partitions), bf16 out ----
            teT = consts.tile([P, DC, TPC], bf16)
            nc.scalar.activation(teT[:], encT_sb[:], ACT.Tanh)
            tpT = consts.tile([P, DC, U], bf16)
            nc.scalar.activation(tpT[:], predT_sb[:], ACT.Tanh)

            # ---- enc_p[t, v] = sum_d teT[d, t] * We[v, d]  (2 halves) ----
            enc_p = consts.tile([TPC, V], bf16)
            for vc in range(NV):
                sl = slice(vc * 512, (vc + 1) * 512)
                eh = ps.tile([TPC, 512], f32, tag="ps", name=f"encps{vc}")
                for c in range(DC):
                    nc.tensor.matmul(eh[:], teT[:, c, :], wt_e[:, c, sl],
                                     start=(c == 0), stop=(c == DC - 1))
                nc.vector.tensor_copy(enc_p[:, sl], eh[:])

            # ---- pred_b[u, v] = sum_d tpT[d, u] * Wp[v, d] + b[v] ----
            movs = [consts.tile([NK, V], bf16, name=f"mov{i}")
                    for i in range(NMOV)]
            for vc in range(NV):
                sl = slice(vc * 512, (vc + 1) * 512)
                ph = ps.tile([U, 512], f32, tag="ps", name=f"predps{vc}")
                for c in range(DC):
                    nc.tensor.matmul(ph[:], tpT[:, c, :], wt_p[:, c, sl],
                                     start=(c == 0), stop=False)
                nc.tensor.matmul(ph[:], ones1[:], b_sb[:, sl],
                                 start=False, stop=True)
                for m in movs:
                    nc.vector.tensor_copy(m[0:U, sl], ph[:])

            # ---- main-loop pieces (emitted out of line so the first
            #      couple of tiles' matmuls can precede the lse chain on
            #      the PE, hiding the lse DMA round-trip latency) ----
            xhalves = {}

            def mm_part(k):
                r0 = k * P
                rows = min(P, ROWS - r0)
                mov = movs[k % NMOV]
                nc.sync.dma_start(out=mov[U:U + 4, :],
                                  in_=enc_p[T0C[k]:T0C[k] + 4, :])
                x0 = ps.tile([P, 512], f32, tag="ps", name="x0")
                nc.tensor.matmul(x0[:rows], stat_sb[:, k, :rows],
                                 mov[:, 0:512], start=True, stop=True)
                x1 = ps.tile([P, 512], f32, tag="ps", name="x1")
                nc.tensor.matmul(x1[:rows], stat_sb[:, k, :rows],
                                 mov[:, 512:V], start=True, stop=True)
                xhalves[k] = (x0, x1, rows, r0)

            def evict_part(k):
                x0, x1, rows, r0 = xhalves.pop(k)
                nb = ncols[:rows, k:k + 1]
                o = outs.tile([P, V], f32, name="o")
                nc.scalar.activation(o[:rows, 0:512], x0[:rows],
                                     ACT.Identity, bias=nb)
                nc.vector.tensor_scalar_add(o[:rows, 512:V], x1[:rows], nb)
                eng = nc.gpsimd if k % 2 == 0 else nc.scalar
                eng.dma_start(out=out.ap()[r0:r0 + rows, :], in_=o[:rows])

            # ---- lse[t, u] = ln(sum_v exp(enc_p) * exp(pred_b)) ----
            Etr = consts.tile([P, VC, TPC], bf16)
            for c in range(VC):
                pt = ps.tile([P, TPC], bf16, tag="ps", name=f"ept{c}")
                nc.tensor.transpose(pt[:], enc_p[:, c * P:(c + 1) * P],
                                    ident_sb[:TPC, :TPC])
                nc.scalar.activation(Etr[:, c, :], pt[:], ACT.Exp)
            Ptr = consts.tile([P, VC, U], bf16)
            for c in range(VC):
                pt = ps.tile([P, U], bf16, tag="ps", name=f"ppt{c}")
                nc.tensor.transpose(pt[:], movs[0][0:U, c * P:(c + 1) * P],
                                    ident_sb[:U, :U])
                nc.scalar.activation(Ptr[:, c, :], pt[:], ACT.Exp)
            # Front-load the first two tiles' matmuls (4 of 8 PSUM bufs;
            # the S-GEMM chain below still has bufs to rotate through).
            NFRONT = 2
            for k in range(NFRONT):
                mm_part(k)
            S_ps = ps.tile([TPC, U], f32, tag="ps")
            for c in range(VC):
                nc.tensor.matmul(S_ps[:], Etr[:, c, :], Ptr[:, c, :],
                                 start=(c == 0), stop=(c == VC - 1))
            lse = consts.tile([TPC, U], f32)
            nc.scalar.activation(lse[:], S_ps[:], ACT.Ln)
            nlse = consts.tile([TPC, U], f32)
            nc.vector.tensor_scalar_mul(nlse[:], lse[:], -1.0)
            # Flatten [100, 50] (row-major = row index r) into column
            # layout [128, 40] via a DRAM round-trip, split across queues
            # so the first half lands early; scratch tail rows are
            # never-read garbage (pad rows only).
            RS = 52 * U                          # flat split at t=52
            nc.sync.dma_start(
                out=scratch.ap()[0:RS].rearrange("(t u) -> t u", t=52),
                in_=nlse[0:52, :])
            nc.gpsimd.dma_start(
                out=scratch.ap()[RS:ROWS].rearrange("(t u) -> t u", t=48),
                in_=nlse[52:TPC, :])
            sc_r = scratch.ap().rearrange("(k p) -> p k", p=P)
            ncols = consts.tile([P, NT], f32)
            nc.scalar.dma_start(out=ncols[:, 0:20], in_=sc_r[:, 0:20])
            nc.gpsimd.dma_start(out=ncols[:, 20:NT], in_=sc_r[:, 20:NT])

            # ---- main loop over row tiles ----
            for k in range(NFRONT):
                evict_part(k)
            for k in range(NFRONT, NT):
                mm_part(k)
                evict_part(k)

    nc.compile()
    return nc


def kernel(enc_out, pred_out, W, b):
    global LAST_RESULT
    enc_out = np.asarray(enc_out, dtype=np.float32)
    pred_out = np.asarray(pred_out, dtype=np.float32)
    W = np.asarray(W, dtype=np.float32)
    b = np.asarray(b, dtype=np.float32)

    if "nc" not in _CACHE:
        _CACHE["nc"] = _build_program()
        _CACHE["stat"] = _build_stationary()
    nc = _CACHE["nc"]

    import ml_dtypes
    wT = np.ascontiguousarray(W.T).astype(ml_dtypes.bfloat16)   # [2D, V]
    wTe = np.ascontiguousarray(wT[:D])
    wTp = np.ascontiguousarray(wT[D:])
    stat = _CACHE["stat"].astype(ml_dtypes.bfloat16)
    ident = np.eye(P, dtype=ml_dtypes.bfloat16)
    bias = np.ascontiguousarray(b.reshape(1, V)).astype(ml_dtypes.bfloat16)
    enc_flat = enc_out.reshape(BT, D)                 # [800, 512]

    in_maps = []
    for c in range(N_CORES):
        bt0 = c * TPC
        b_idx = bt0 // T
        in_maps.append({
            "encT": np.ascontiguousarray(enc_flat[bt0:bt0 + TPC].T),
            "predT": np.ascontiguousarray(pred_out[b_idx].T),
            "wTe": wTe,
            "wTp": wTp,
            "bias": bias,
            "stat": stat,
            "ident": ident,
        })

    res = run_bass_kernel_spmd(nc, in_maps, core_ids=list(range(N_CORES)),
                               trace=TRACE)
    LAST_RESULT = res
    full = np.concatenate([r["out"] for r in res.results], axis=0)
    return full.reshape(B, T, U, V)
